# revision 1
# baseline (speedup 1.0000x reference)
"""Trainium2 Bass kernel for nn_CMCI_Mamba.

Strategy: data-parallel over the 2B=8 mamba streams (1 sequence per core).
Each launch runs 2 chained mamba layers fully on-chip in d-major layout
(features on partitions, time on the free axis); the SSM recurrence uses the
DVE tensor_tensor_scan instruction (state = dA*state + dBu along free dim),
one scan per SSM state index s (16 scans of a (128, 2048) tile per layer).
B_s/C_s row-broadcasts are fp32r PE matmuls with a stride-0 (column-
broadcast) lhsT reading xact directly; s-loop elementwise muls run in fp16
(DVE 2x mode). Host does the cheap cross-stream elementwise combines
between the 2 launches.
"""
import sys
import numpy as np
from contextlib import ExitStack

for _p in ("/opt/trn_rl_repo",):
    if _p not in sys.path:
        sys.path.insert(0, _p)

import concourse.bass as bass
import concourse.bacc as bacc
import concourse.tile as tile
from concourse import mybir
from concourse import bass_utils

T, DM, DI, DS, DR, K, NL = 2048, 64, 128, 16, 4, 4, 2
B, C = 4, 2048
FP = mybir.dt.float32
FR = mybir.dt.float32r
FH = mybir.dt.float16
AX = mybir.AluOpType
AF = mybir.ActivationFunctionType

# param blob column layout (blob is (128, 512) fp32 per layer)
_B_INW = 0        # [0:64, 0:256]    in_wT
_B_CONVW = 256    # [:, 256:260]     conv_w
_B_CONVB = 260    # [:, 260]         conv_b
_B_XPW = 261      # [:, 261:297]     xp_wT
_B_DTW = 297      # [0:4, 297:425]   dt_wT
_B_DTB = 425      # [:, 425]         dt_b
_B_ANEG = 426     # [:, 426:442]     -exp(A_log)
_B_D = 442        # [:, 442]         D
_B_OUTW = 443     # [:, 443:507]     out_wT
_BLOB_W = 512


def _pack_blob(raw, l):
    blob = np.zeros((DI, _BLOB_W), np.float32)
    blob[:DM, _B_INW:_B_INW + 2 * DI] = raw["in_w"][l].T
    blob[:, _B_CONVW:_B_CONVW + K] = raw["conv_w"][l]
    blob[:, _B_CONVB] = raw["conv_b"][l]
    blob[:, _B_XPW:_B_XPW + DR + 2 * DS] = raw["xp_w"][l].T
    blob[:DR, _B_DTW:_B_DTW + DI] = raw["dt_w"][l].T
    blob[:, _B_DTB] = raw["dt_b"][l]
    blob[:, _B_ANEG:_B_ANEG + DS] = -np.exp(raw["A_log"][l])
    blob[:, _B_D] = raw["D"][l]
    blob[:, _B_OUTW:_B_OUTW + DM] = raw["out_w"][l].T
    return blob


def _build_kernel(ctx, tc, u0T, blobs, outs):
    nc = tc.nc
    NCH = 4
    CF = T // NCH  # 512 free elems per matmul (one PSUM bank)

    const = ctx.enter_context(tc.tile_pool(name="const", bufs=1))
    big = ctx.enter_context(tc.tile_pool(name="big", bufs=1))
    ub = ctx.enter_context(tc.tile_pool(name="ub", bufs=2))
    sl = ctx.enter_context(tc.tile_pool(name="sl", bufs=3))
    ps = ctx.enter_context(tc.tile_pool(name="ps", bufs=4, space="PSUM"))

    pb = []
    for l in range(NL):
        t = const.tile([DI, _BLOB_W], FP, tag=f"pb{l}", name=f"pb{l}")
        nc.sync.dma_start(t[:], blobs[l][:])
        pb.append(t)

    u_t = ub.tile([DM, T], FP, tag="u", name="u_in")
    nc.sync.dma_start(u_t[:], u0T[:])

    for l in range(NL):
        p = pb[l]
        in_wT = p[0:DM, _B_INW:_B_INW + 2 * DI]
        convw = p[:, _B_CONVW:_B_CONVW + K]
        convb = p[:, _B_CONVB:_B_CONVB + 1]
        xp_wT = p[:, _B_XPW:_B_XPW + DR + 2 * DS]
        dt_wT = p[0:DR, _B_DTW:_B_DTW + DI]
        dt_b = p[:, _B_DTB:_B_DTB + 1]
        Aneg = p[:, _B_ANEG:_B_ANEG + DS]
        Dvec = p[:, _B_D:_B_D + 1]
        out_wT = p[:, _B_OUTW:_B_OUTW + DM]

        # rounded copies for the fp32r broadcast matmuls
        xpw_r = big.tile([DI, DR + 2 * DS], FR, tag="xpw_r", name=f"xpwr{l}")
        nc.vector.tensor_copy(xpw_r[:], xp_wT)

        xpad = big.tile([DI, T + K - 1], FP, tag="xpad", name=f"xpad{l}")
        zs = big.tile([DI, T], FH, tag="zs", name=f"zs{l}")
        ztmp = big.tile([DI, T], FH, tag="ztmp", name=f"ztmp{l}")
        nc.gpsimd.memset(xpad[:, 0:K - 1], 0.0)
        for c in range(NCH):
            cs = slice(c * CF, (c + 1) * CF)
            mm = ps.tile([DI, CF], FP, tag="mm", name=f"mmx{l}_{c}")
            nc.tensor.matmul(mm[:], in_wT[:, 0:DI], u_t[:, cs],
                             start=True, stop=True)
            nc.scalar.activation(xpad[:, K - 1 + c * CF:K - 1 + (c + 1) * CF],
                                 mm[:], AF.Copy)
            mm2 = ps.tile([DI, CF], FP, tag="mm", name=f"mmz{l}_{c}")
            nc.tensor.matmul(mm2[:], in_wT[:, DI:2 * DI], u_t[:, cs],
                             start=True, stop=True)
            nc.scalar.activation(zs[:, cs], mm2[:], AF.Sigmoid)
            nc.scalar.activation(ztmp[:, cs], mm2[:], AF.Copy)
        # zs = z * sigmoid(z)  (fp16 2x, and off the critical DVE path)
        nc.vector.tensor_mul(zs[:], zs[:], ztmp[:])

        # causal depthwise conv along t (shifts are free-axis offsets);
        # conv_b folded via the two-scalar tensor_scalar form. Chunked so
        # the chain starts as soon as the first xpad chunk lands instead of
        # waiting for the full row (kills a ~16us DVE ramp per layer).
        xconv = big.tile([DI, T], FP, tag="xconv", name=f"xconv{l}")
        xact = big.tile([DI, T], FP, tag="xact", name=f"xact{l}")
        xsg = sl.tile([DI, T], FP, tag="dA", name=f"xsg{l}")
        xact_r = big.tile([DI, T], FR, tag="xact_r", name=f"xactr{l}")
        for c in range(NCH):
            cs = slice(c * CF, (c + 1) * CF)
            base = c * CF
            nc.vector.tensor_scalar(xconv[:, cs],
                                    xpad[:, K - 1 + base:K - 1 + base + CF],
                                    convw[:, K - 1:K], convb,
                                    AX.mult, AX.add)
            for k in range(K - 1):
                nc.vector.scalar_tensor_tensor(
                    xconv[:, cs], xpad[:, k + base:k + base + CF],
                    convw[:, k:k + 1], xconv[:, cs], AX.mult, AX.add)
            # xact = xconv * sigmoid(xconv)
            nc.scalar.activation(xsg[:, cs], xconv[:, cs], AF.Sigmoid)
            nc.vector.tensor_mul(xact[:, cs], xconv[:, cs], xsg[:, cs])
            nc.vector.tensor_copy(xact_r[:, cs], xact[:, cs])

        # dt rows of the x-projection (only rows 0:4 are needed in SBUF;
        # B/C rows are recomputed by the broadcast matmuls)
        dtT = big.tile([DR, T], FP, tag="dtT", name=f"dtT{l}")  # shares "dtT" tag with yf below
        for c in range(NCH):
            cs = slice(c * CF, (c + 1) * CF)
            mm = ps.tile([DI, CF], FP, tag="mm", name=f"mmp{l}_{c}")
            nc.tensor.matmul(mm[0:DR, :], xp_wT[:, 0:DR], xact[:, cs],
                             start=True, stop=True)
            nc.scalar.activation(dtT[:, cs], mm[0:DR, :], AF.Copy)

        # softplus(v) = ln(1 + exp(v)), v = dtproj + dt_b (|v| stays far
        # from fp32 exp overflow for this model's data distribution)
        delta = big.tile([DI, T], FP, tag="delta", name=f"delta{l}")
        ev = big.tile([DI, T + K - 1], FP, tag="xpad", name=f"ev{l}")
        for c in range(NCH):
            cs = slice(c * CF, (c + 1) * CF)
            mm = ps.tile([DI, CF], FP, tag="mm", name=f"mmd{l}_{c}")
            nc.tensor.matmul(mm[:], dt_wT[:], dtT[:, cs],
                             start=True, stop=True)
            nc.scalar.activation(ev[:, cs], mm[:], AF.Exp, bias=dt_b)
        dx16 = big.tile([DI, T], FH, tag="dx16", name=f"dx16_{l}")
        for c in range(NCH):
            cs = slice(c * CF, (c + 1) * CF)
            nc.scalar.activation(delta[:, cs], ev[:, cs], AF.Ln, bias=1.0)
            nc.vector.tensor_mul(dx16[:, cs], delta[:, cs], xact[:, cs])

        yacc = big.tile([DI, T], FH, tag="yacc", name=f"yacc{l}")
        for s in range(DS):
            dA = sl.tile([DI, T], FP, tag="dA", name=f"dA{l}_{s}")
            nc.scalar.activation(dA[:], delta[:], AF.Exp,
                                 scale=Aneg[:, s:s + 1])
            # B_s broadcast: out[p,t] = sum_d xp_w[4+s,d] * xact[d,t]
            # (two 1024-wide halves so bc PSUM rotates through the shared
            # 2-bank slots instead of serializing on one 4-bank tile)
            bcol = xpw_r[:, DR + s:DR + s + 1].broadcast_to((DI, DI))
            brep16 = sl.tile([DI, T], FH, tag="brep", name=f"brep{l}_{s}")
            for hf in range(2):
                hs_ = slice(hf * 1024, (hf + 1) * 1024)
                bps = ps.tile([DI, 1024], FP, tag="mm", name=f"bps{l}_{s}_{hf}")
                for c in range(2):
                    cs = slice(c * CF, (c + 1) * CF)
                    nc.tensor.matmul(bps[:, cs], bcol,
                                     xact_r[:, hf * 1024 + c * CF:
                                            hf * 1024 + (c + 1) * CF],
                                     start=True, stop=True)
                nc.scalar.activation(brep16[:, hs_], bps[:], AF.Copy)
            dBu16 = sl.tile([DI, T], FH, tag="dBu", name=f"dBu{l}_{s}")
            nc.vector.tensor_mul(dBu16[:], dx16[:], brep16[:])
            hs16 = sl.tile([DI, T], FH, tag="hs", name=f"hs{l}_{s}")
            nc.vector.tensor_tensor_scan(hs16[:], dA[:], dBu16[:], 0.0,
                                         AX.mult, AX.add)
            ccol = xpw_r[:, DR + DS + s:DR + DS + s + 1].broadcast_to((DI, DI))
            crep16 = sl.tile([DI, T], FH, tag="crep", name=f"crep{l}_{s}")
            for hf in range(2):
                hs_ = slice(hf * 1024, (hf + 1) * 1024)
                cps = ps.tile([DI, 1024], FP, tag="mm", name=f"cps{l}_{s}_{hf}")
                for c in range(2):
                    cs = slice(c * CF, (c + 1) * CF)
                    nc.tensor.matmul(cps[:, cs], ccol,
                                     xact_r[:, hf * 1024 + c * CF:
                                            hf * 1024 + (c + 1) * CF],
                                     start=True, stop=True)
                nc.scalar.activation(crep16[:, hs_], cps[:], AF.Copy)
            if s == 0:
                nc.vector.tensor_mul(yacc[:], hs16[:], crep16[:])
            else:
                hsc16 = sl.tile([DI, T], FH, tag="hsc", name=f"hsc{l}_{s}")
                nc.vector.tensor_mul(hsc16[:], hs16[:], crep16[:])
                nc.vector.tensor_add(yacc[:], yacc[:], hsc16[:])

        # y = yacc + D*x ; y *= silu(z)
        yf = big.tile([DI, T], FP, tag="dtT", name=f"yf{l}")
        nc.vector.scalar_tensor_tensor(yf[:], xact[:], Dvec, yacc[:],
                                       AX.mult, AX.add)
        nc.vector.tensor_mul(yf[:], yf[:], zs[:])

        o_t = ub.tile([DM, T], FP, tag="u", name=f"o{l}")
        for c in range(NCH):
            cs = slice(c * CF, (c + 1) * CF)
            mm = ps.tile([DI, CF], FP, tag="mm", name=f"mmo{l}_{c}")
            nc.tensor.matmul(mm[0:DM, :], out_wT[:], yf[:, cs],
                             start=True, stop=True)
            nc.scalar.activation(o_t[:, cs], mm[0:DM, :], AF.Copy)
        nc.sync.dma_start(outs[l][:], o_t[:])
        u_t = o_t


def build_program():
    nc = bacc.Bacc("TRN2", target_bir_lowering=False, debug=False)
    u0T = nc.dram_tensor("u0T", [DM, T], FP, kind="ExternalInput").ap()
    blobs = [nc.dram_tensor(f"pblob_l{l}", [DI, _BLOB_W], FP,
                            kind="ExternalInput").ap() for l in range(NL)]
    outs = [nc.dram_tensor(f"o{l + 1}T", [DM, T], FP,
                           kind="ExternalOutput").ap() for l in range(NL)]
    with tile.TileContext(nc) as tc:
        with ExitStack() as ctx:
            _build_kernel(ctx, tc, u0T, blobs, outs)
    nc.compile()
    return nc


_PROG = None


def _get_prog():
    global _PROG
    if _PROG is None:
        _PROG = build_program()
    return _PROG


def _run_launch(u_list_T, raw, trace=False, trace_kwargs=None):
    """u_list_T: list of 8 arrays (64, 2048) f32. raw: param dict (np).
    Returns (o1_list, o2_list) of (64, 2048) arrays, and the raw result."""
    nc = _get_prog()
    blobs = [_pack_blob(raw, l) for l in range(NL)]
    in_maps = []
    for b in range(8):
        in_maps.append({
            "u0T": np.ascontiguousarray(u_list_T[b], np.float32),
            "pblob_l0": blobs[0],
            "pblob_l1": blobs[1],
        })
    res = bass_utils.run_bass_kernel_spmd(
        nc, in_maps, core_ids=list(range(8)), trace=trace,
        **(trace_kwargs or {}))
    o1 = [res.results[b]["o1T"] for b in range(8)]
    o2 = [res.results[b]["o2T"] for b in range(8)]
    return o1, o2, res


def kernel(**inputs):
    inp = {k: np.asarray(v, np.float32) for k, v in inputs.items()}
    Ms = inp["Ms_feature"]
    Pan = inp["Pan_feature"]
    h = C // 2
    rawa = {n: inp["a_" + n] for n in ("in_w", "conv_w", "conv_b", "xp_w",
                                       "dt_w", "dt_b", "A_log", "D", "out_w")}
    rawb = {n: inp["b_" + n] for n in ("in_w", "conv_w", "conv_b", "xp_w",
                                       "dt_w", "dt_b", "A_log", "D", "out_w")}

    cf1 = np.concatenate([Ms[:, :h], Pan[:, h:]], axis=1)
    cf2 = np.concatenate([Pan[:, :h], Ms[:, h:]], axis=1)
    u_list = [cf1[b].T for b in range(B)] + [cf2[b].T for b in range(B)]
    o1, o2, _ = _run_launch(u_list, rawa)
    cf1_1 = np.stack([o1[b].T for b in range(B)])
    cf2_1 = np.stack([o1[B + b].T for b in range(B)])
    cf1_2 = np.stack([o2[b].T for b in range(B)])
    cf2_2 = np.stack([o2[B + b].T for b in range(B)])
    Ms1 = np.maximum((cf1_1 + cf2_1) * 0.5 + Ms, 0.0)
    Ms2 = np.maximum((cf1_2 + cf2_2) * 0.5 + Ms1, 0.0)

    cf3 = np.stack([Pan[:, ::2], Ms2[:, 1::2]], axis=2).reshape(B, C, DM)
    cf4 = np.stack([Ms2[:, ::2], Pan[:, 1::2]], axis=2).reshape(B, C, DM)
    u_list = [cf3[b].T for b in range(B)] + [cf4[b].T for b in range(B)]
    o1, o2, _ = _run_launch(u_list, rawb)
    cf3_1 = np.stack([o1[b].T for b in range(B)])
    cf4_1 = np.stack([o1[B + b].T for b in range(B)])
    cf3_2 = np.stack([o2[b].T for b in range(B)])
    cf4_2 = np.stack([o2[B + b].T for b in range(B)])
    Pan1 = np.maximum((cf3_1 + cf4_1) * 0.5 + Pan, 0.0)
    Pan2 = np.maximum((cf3_2 + cf4_2) * 0.5 + Pan1, 0.0)
    return Ms2, Pan2



# revision 2
# speedup vs baseline: 1.0203x; 1.0203x over previous
"""Trainium2 Bass kernel for nn_CMCI_Mamba — v2 (engine-rebalanced).

Data-parallel over the 2B=8 mamba streams (1 per core), 2 chained layers
per launch, d-major layout (d_inner on partitions, time on free axis).

v2 vs baseline:
- fp16 tiles everywhere (host pre-casts inputs / params, upcasts outputs)
- silu fused into ACT Swish evictions (no sigmoid+mul pairs)
- depthwise conv on PE via diagonal-lhsT accumulated matmuls
- x-projection computed once (all 36 rows); per-state B/C row broadcasts
  are one-hot selection matmuls reading the 36-row tile
- per-state products on DVE; the sum over states runs on PE as
  identity-matmul PSUM accumulation
- scans paired: one tensor_tensor_scan of FD=4096 covers 2 states
  (decay column at the segment boundary zeroed to reset the recurrence)
"""
import sys
import numpy as np
from contextlib import ExitStack

for _p in ("/opt/trn_rl_repo",):
    if _p not in sys.path:
        sys.path.insert(0, _p)

import concourse.bass as bass
import concourse.bacc as bacc
import concourse.tile as tile
from concourse import mybir
from concourse import bass_utils

T, DM, DI, DS, DR, K, NL = 2048, 64, 128, 16, 4, 4, 2
B, C = 4, 2048
XR = DR + 2 * DS  # 36 rows of the x-projection
FP = mybir.dt.float32
FH = mybir.dt.float16
AX = mybir.AluOpType
AF = mybir.ActivationFunctionType

# fp16 param blob column layout, (128, _FW) per layer
_F_INW = 0                    # [0:64, 0:256]   in_wT (x cols 0:128, z cols 128:256)
_F_DIAG = 256                 # [:, 256:768]    conv diag_k (4 x 128)
_F_XPW = 768                  # [0:128, 768:804]  xp_wT (128, 36)
_F_DTW = 804                  # [0:4, 804:932]  dt_wT (4, 128)
_F_OUTW = 932                 # [:, 932:996]    out_wT (128, 64)
_F_OH = 996                   # one-hot B/C selectors for the MM-path pairs
_N_MM_PAIRS = 0               # pairs < this broadcast via PE+ACT, rest via DMA
_F_ID = _F_OH + 4 * _N_MM_PAIRS * 128
_FW = _F_ID + 128

# fp32 vector blob (128, 19): conv_b, dt_b, D, Aneg[16]
_V_CONVB, _V_DTB, _V_D, _V_ANEG = 0, 1, 2, 3
_VW = 19


def _pack_fh(raw, l):
    fh = np.zeros((DI, _FW), np.float16)
    fh[:DM, _F_INW:_F_INW + 2 * DI] = raw["in_w"][l].T
    for k in range(K):
        d0 = _F_DIAG + k * DI
        fh[:, d0:d0 + DI][np.arange(DI), np.arange(DI)] = raw["conv_w"][l][:, k]
    fh[:, _F_XPW:_F_XPW + XR] = raw["xp_w"][l].T
    fh[:DR, _F_DTW:_F_DTW + DI] = raw["dt_w"][l].T
    fh[:, _F_OUTW:_F_OUTW + DM] = raw["out_w"][l].T
    for p in range(_N_MM_PAIRS):
        for r in range(4):  # B(s0), B(s1), C(s0), C(s1)
            row = DR + 2 * p + (r & 1) + (DS if r >= 2 else 0)
            c0 = _F_OH + (4 * p + r) * DI
            fh[row, c0:c0 + DI] = 1.0
    fh[:, _F_ID:_F_ID + DI][np.arange(DI), np.arange(DI)] = 1.0
    return fh


def _pack_fv(raw, l):
    fv = np.zeros((DI, _VW), np.float32)
    fv[:, _V_CONVB] = raw["conv_b"][l]
    fv[:, _V_DTB] = raw["dt_b"][l]
    fv[:, _V_D] = raw["D"][l]
    fv[:, _V_ANEG:_V_ANEG + DS] = -np.exp(raw["A_log"][l])
    return fv


def _build_kernel(ctx, tc, u0T, fhs, fvs, xdbl_ds, outs):
    nc = tc.nc
    NCH = 4
    CF = T // NCH  # 512

    const = ctx.enter_context(tc.tile_pool(name="const", bufs=1))
    big = ctx.enter_context(tc.tile_pool(name="big", bufs=1))
    ub = ctx.enter_context(tc.tile_pool(name="ub", bufs=2))
    sl = ctx.enter_context(tc.tile_pool(name="sl", bufs=2))
    bc = ctx.enter_context(tc.tile_pool(name="bc", bufs=4))
    ps1 = ctx.enter_context(tc.tile_pool(name="ps1", bufs=4, space="PSUM"))
    psy = ctx.enter_context(tc.tile_pool(name="psy", bufs=1, space="PSUM"))

    fh = []
    fv = []
    for l in range(NL):
        t = const.tile([DI, _FW], FH, tag=f"fh{l}", name=f"fh{l}")
        nc.sync.dma_start(t[:], fhs[l][:])
        fh.append(t)
        v = const.tile([DI, _VW], FP, tag=f"fv{l}", name=f"fv{l}")
        nc.sync.dma_start(v[:], fvs[l][:])
        fv.append(v)

    u_t = ub.tile([DM, T], FH, tag="u", name="u_in")
    nc.sync.dma_start(u_t[:], u0T[:])

    for l in range(NL):
        h = fh[l]
        v = fv[l]
        in_wT = h[0:DM, _F_INW:_F_INW + 2 * DI]
        xp_wT = h[0:DI, _F_XPW:_F_XPW + XR]
        dt_wT = h[0:DR, _F_DTW:_F_DTW + DI]
        out_wT = h[:, _F_OUTW:_F_OUTW + DM]
        ident = h[:, _F_ID:_F_ID + DI]
        convb = v[:, _V_CONVB:_V_CONVB + 1]
        dt_b = v[:, _V_DTB:_V_DTB + 1]
        Dvec = v[:, _V_D:_V_D + 1]

        # ---- stage 1: projections + conv ----
        xpre = big.tile([DI, T + K - 1], FH, tag="xpre", name=f"xpre{l}")
        zs = big.tile([DI, T], FH, tag="zs", name=f"zs{l}")
        xact = big.tile([DI, T], FH, tag="xact", name=f"xact{l}")
        xdbl = big.tile([XR, T], FH, tag="xdbl", name=f"xdbl{l}")
        ev = big.tile([DI, T], FP, tag="ev", name=f"ev{l}")
        delta16 = big.tile([DI, T], FH, tag="delta", name=f"delta{l}")
        dx16 = big.tile([DI, T], FH, tag="dx", name=f"dx{l}")
        nc.gpsimd.memset(xpre[:, 0:K - 1], 0.0)
        for c in range(NCH):
            cs = slice(c * CF, (c + 1) * CF)
            mmx = ps1.tile([DI, CF], FP, tag="mm", name=f"mmx{l}_{c}")
            nc.tensor.matmul(mmx[:], in_wT[:, 0:DI], u_t[:, cs],
                             start=True, stop=True)
            nc.scalar.activation(xpre[:, K - 1 + c * CF:K - 1 + (c + 1) * CF],
                                 mmx[:], AF.Copy)
            mmz = ps1.tile([DI, CF], FP, tag="mm", name=f"mmz{l}_{c}")
            nc.tensor.matmul(mmz[:], in_wT[:, DI:2 * DI], u_t[:, cs],
                             start=True, stop=True)
            nc.scalar.activation(zs[:, cs], mmz[:], AF.Silu)
        for c in range(NCH):
            cs = slice(c * CF, (c + 1) * CF)
            cps = ps1.tile([DI, CF], FP, tag="mm", name=f"cps{l}_{c}")
            for k in range(K):
                nc.tensor.matmul(cps[:], h[:, _F_DIAG + k * DI:_F_DIAG + (k + 1) * DI],
                                 xpre[:, k + c * CF:k + c * CF + CF],
                                 start=(k == 0), stop=(k == K - 1))
            nc.scalar.activation(xact[:, cs], cps[:], AF.Silu, bias=convb)
        for c in range(NCH):
            cs = slice(c * CF, (c + 1) * CF)
            mmp = ps1.tile([DI, CF], FP, tag="mm", name=f"mmp{l}_{c}")
            nc.tensor.matmul(mmp[0:XR, :], xp_wT, xact[:, cs],
                             start=True, stop=True)
            nc.scalar.activation(xdbl[:, cs], mmp[0:XR, :], AF.Copy)
        # stage the B/C rows in DRAM, pair-interleaved [B2p B2p+1 C2p C2p+1]
        nc.sync.dma_start(xdbl_ds[l][:, 0:2 * T], xdbl[DR:DR + DS, :])
        nc.sync.dma_start(xdbl_ds[l][:, 2 * T:4 * T], xdbl[DR + DS:XR, :])
        for c in range(NCH):
            cs = slice(c * CF, (c + 1) * CF)
            mmd = ps1.tile([DI, CF], FP, tag="mm", name=f"mmd{l}_{c}")
            nc.tensor.matmul(mmd[:], dt_wT, xdbl[0:DR, cs],
                             start=True, stop=True)
            nc.scalar.activation(ev[:, cs], mmd[:], AF.Exp, bias=dt_b)
        nc.scalar.activation(delta16[:], ev[:], AF.Ln, bias=1.0)
        nc.vector.tensor_mul(dx16[:], delta16[:], xact[:])

        # ---- SSM: 8 pairs of states ----
        y_ps = psy.tile([DI, T], FP, tag="y", name=f"y{l}")
        for p in range(DS // 2):
            s0 = 2 * p
            # bcrep layout: [B(s0) | B(s0+1) | C(s0) | C(s0+1)], T cols each
            bcrep = bc.tile([DI, 4 * T], FH, tag="bcrep", name=f"bcrep{l}_{p}")
            if p < _N_MM_PAIRS:
                for r in range(4):
                    oh = h[0:XR, _F_OH + (4 * p + r) * DI:
                           _F_OH + (4 * p + r + 1) * DI]
                    for c in range(NCH):
                        bps = ps1.tile([DI, CF], FP, tag="mm",
                                       name=f"bc{l}_{p}_{r}_{c}")
                        nc.tensor.matmul(bps[:], oh,
                                         xdbl[:, c * CF:(c + 1) * CF],
                                         start=True, stop=True)
                        nc.scalar.activation(
                            bcrep[:, r * T + c * CF:r * T + (c + 1) * CF],
                            bps[:], AF.Copy)
            else:
                src = xdbl_ds[l][p:p + 1, :]
                nc.sync.dma_start(bcrep[:], src.broadcast_to((DI, 4 * T)))
            brep = bcrep[:, 0:2 * T]
            crep = bcrep[:, 2 * T:4 * T]
            dA2 = sl.tile([DI, 2 * T], FH, tag="dA2", name=f"dA2{l}_{p}")
            for q in range(2):
                nc.scalar.activation(dA2[:, q * T:(q + 1) * T], delta16[:],
                                     AF.Exp,
                                     scale=v[:, _V_ANEG + s0 + q:_V_ANEG + s0 + q + 1])
            nc.gpsimd.memset(dA2[:, T:T + 1], 0.0)
            dBu2 = sl.tile([DI, 2 * T], FH, tag="dBu2", name=f"dBu2{l}_{p}")
            nc.vector.tensor_mul(dBu2[:, 0:T], dx16[:], brep[:, 0:T])
            nc.vector.tensor_mul(dBu2[:, T:2 * T], dx16[:], brep[:, T:2 * T])
            hs2 = sl.tile([DI, 2 * T], FH, tag="hs2", name=f"hs2{l}_{p}")
            nc.vector.tensor_tensor_scan(hs2[:], dA2[:], dBu2[:], 0.0,
                                         AX.mult, AX.add)
            hsC2 = sl.tile([DI, 2 * T], FH, tag="hsC2", name=f"hsC2{l}_{p}")
            nc.vector.tensor_mul(hsC2[:], hs2[:], crep[:])
            for q in range(2):
                for c in range(NCH):
                    nc.tensor.matmul(y_ps[:, c * CF:(c + 1) * CF], ident,
                                     hsC2[:, q * T + c * CF:q * T + (c + 1) * CF],
                                     start=(p == 0 and q == 0),
                                     stop=(p == DS // 2 - 1 and q == 1),
                                     skip_group_check=True)

        # ---- tail: y = (yacc + D*x) * silu(z); out projection ----
        y16 = big.tile([DI, T], FH, tag="y16", name=f"y16{l}")
        nc.scalar.activation(y16[:], y_ps[:], AF.Copy)
        ydx = big.tile([DI, T], FH, tag="ydx", name=f"ydx{l}")
        nc.vector.tensor_scalar_mul(ydx[:], xact[:], Dvec)
        nc.vector.tensor_add(ydx[:], ydx[:], y16[:])
        yf = big.tile([DI, T], FH, tag="yf", name=f"yf{l}")
        nc.vector.tensor_mul(yf[:], ydx[:], zs[:])

        o_t = ub.tile([DM, T], FH, tag="u", name=f"o{l}")
        for c in range(NCH):
            cs = slice(c * CF, (c + 1) * CF)
            omm = ps1.tile([DM, CF], FP, tag="mm", name=f"omm{l}_{c}")
            nc.tensor.matmul(omm[:], out_wT, yf[:, cs], start=True, stop=True)
            nc.scalar.activation(o_t[:, cs], omm[:], AF.Copy)
        nc.sync.dma_start(outs[l][:], o_t[:])
        u_t = o_t


def build_program():
    nc = bacc.Bacc("TRN2", target_bir_lowering=False, debug=False)
    u0T = nc.dram_tensor("u0T", [DM, T], FH, kind="ExternalInput").ap()
    fhs = [nc.dram_tensor(f"fh_l{l}", [DI, _FW], FH,
                          kind="ExternalInput").ap() for l in range(NL)]
    fvs = [nc.dram_tensor(f"fv_l{l}", [DI, _VW], FP,
                          kind="ExternalInput").ap() for l in range(NL)]
    xdbl_ds = [nc.dram_tensor(f"xdbl_d{l}", [DS // 2, 4 * T], FH,
                              kind="Internal").ap() for l in range(NL)]
    outs = [nc.dram_tensor(f"o{l + 1}T", [DM, T], FH,
                           kind="ExternalOutput").ap() for l in range(NL)]
    with tile.TileContext(nc) as tc:
        with ExitStack() as ctx:
            _build_kernel(ctx, tc, u0T, fhs, fvs, xdbl_ds, outs)
    nc.compile()
    return nc


_PROG = None


def _get_prog():
    global _PROG
    if _PROG is None:
        _PROG = build_program()
    return _PROG


def _run_launch(u_list_T, raw, trace=False, trace_kwargs=None):
    """u_list_T: list of 8 arrays (64, 2048) fp16. raw: param dict (np).
    Returns (o1_list, o2_list) fp16 arrays and the raw result."""
    nc = _get_prog()
    fhs = [_pack_fh(raw, l) for l in range(NL)]
    fvs = [_pack_fv(raw, l) for l in range(NL)]
    in_maps = []
    for b in range(8):
        in_maps.append({
            "u0T": np.ascontiguousarray(u_list_T[b], np.float16),
            "fh_l0": fhs[0], "fh_l1": fhs[1],
            "fv_l0": fvs[0], "fv_l1": fvs[1],
        })
    res = bass_utils.run_bass_kernel_spmd(
        nc, in_maps, core_ids=list(range(8)), trace=trace,
        **(trace_kwargs or {}))
    o1 = [res.results[b]["o1T"] for b in range(8)]
    o2 = [res.results[b]["o2T"] for b in range(8)]
    return o1, o2, res


def kernel(**inputs):
    inp = {k: np.asarray(v, np.float32) for k, v in inputs.items()}
    Ms = inp["Ms_feature"]
    Pan = inp["Pan_feature"]
    h = C // 2
    names = ("in_w", "conv_w", "conv_b", "xp_w", "dt_w", "dt_b",
             "A_log", "D", "out_w")
    rawa = {n: inp["a_" + n] for n in names}
    rawb = {n: inp["b_" + n] for n in names}

    cf1 = np.concatenate([Ms[:, :h], Pan[:, h:]], axis=1)
    cf2 = np.concatenate([Pan[:, :h], Ms[:, h:]], axis=1)
    u_list = [cf1[b].T for b in range(B)] + [cf2[b].T for b in range(B)]
    o1, o2, _ = _run_launch(u_list, rawa)
    cf1_1 = np.stack([o1[b].T.astype(np.float32) for b in range(B)])
    cf2_1 = np.stack([o1[B + b].T.astype(np.float32) for b in range(B)])
    cf1_2 = np.stack([o2[b].T.astype(np.float32) for b in range(B)])
    cf2_2 = np.stack([o2[B + b].T.astype(np.float32) for b in range(B)])
    Ms1 = np.maximum((cf1_1 + cf2_1) * 0.5 + Ms, 0.0)
    Ms2 = np.maximum((cf1_2 + cf2_2) * 0.5 + Ms1, 0.0)

    cf3 = np.stack([Pan[:, ::2], Ms2[:, 1::2]], axis=2).reshape(B, C, DM)
    cf4 = np.stack([Ms2[:, ::2], Pan[:, 1::2]], axis=2).reshape(B, C, DM)
    u_list = [cf3[b].T for b in range(B)] + [cf4[b].T for b in range(B)]
    o1, o2, _ = _run_launch(u_list, rawb)
    cf3_1 = np.stack([o1[b].T.astype(np.float32) for b in range(B)])
    cf4_1 = np.stack([o1[B + b].T.astype(np.float32) for b in range(B)])
    cf3_2 = np.stack([o2[b].T.astype(np.float32) for b in range(B)])
    cf4_2 = np.stack([o2[B + b].T.astype(np.float32) for b in range(B)])
    Pan1 = np.maximum((cf3_1 + cf4_1) * 0.5 + Pan, 0.0)
    Pan2 = np.maximum((cf3_2 + cf4_2) * 0.5 + Pan1, 0.0)
    return Ms2, Pan2


# revision 3
# speedup vs baseline: 1.0209x; 1.0006x over previous
"""Trainium2 Bass kernel for nn_CMCI_Mamba — v2 (engine-rebalanced).

Data-parallel over the 2B=8 mamba streams (1 per core), 2 chained layers
per launch, d-major layout (d_inner on partitions, time on free axis).

v2 vs baseline:
- fp16 tiles everywhere (host pre-casts inputs / params, upcasts outputs)
- silu fused into ACT Swish evictions (no sigmoid+mul pairs)
- depthwise conv on PE via diagonal-lhsT accumulated matmuls
- x-projection computed once (all 36 rows); per-state B/C row broadcasts
  are one-hot selection matmuls reading the 36-row tile
- per-state products on DVE; the sum over states runs on PE as
  identity-matmul PSUM accumulation
- scans paired: one tensor_tensor_scan of FD=4096 covers 2 states
  (decay column at the segment boundary zeroed to reset the recurrence)
"""
import sys
import numpy as np
from contextlib import ExitStack

for _p in ("/opt/trn_rl_repo",):
    if _p not in sys.path:
        sys.path.insert(0, _p)

import concourse.bass as bass
import concourse.bacc as bacc
import concourse.tile as tile
from concourse import mybir
from concourse import bass_utils

T, DM, DI, DS, DR, K, NL = 2048, 64, 128, 16, 4, 4, 2
B, C = 4, 2048
XR = DR + 2 * DS  # 36 rows of the x-projection
FP = mybir.dt.float32
FH = mybir.dt.float16
AX = mybir.AluOpType
AF = mybir.ActivationFunctionType

# fp16 param blob column layout, (128, _FW) per layer
_F_INW = 0                    # [0:64, 0:256]   in_wT (x cols 0:128, z cols 128:256)
_F_DIAG = 256                 # [:, 256:768]    conv diag_k (4 x 128)
_F_XPW = 768                  # [0:128, 768:804]  xp_wT (128, 36)
_F_DTW = 804                  # [0:4, 804:932]  dt_wT (4, 128)
_F_OUTW = 932                 # [:, 932:996]    out_wT (128, 64)
_F_OH = 996                   # one-hot B/C selectors for the MM-path pairs
_N_MM_PAIRS = 0               # pairs < this broadcast via PE+ACT, rest via DMA
_F_ID = _F_OH + 4 * _N_MM_PAIRS * 128
_FW = _F_ID + 128

# fp32 vector blob (128, 19): conv_b, dt_b, D, Aneg[16]
_V_CONVB, _V_DTB, _V_D, _V_ANEG = 0, 1, 2, 3
_VW = 19


def _pack_fh(raw, l):
    fh = np.zeros((DI, _FW), np.float16)
    fh[:DM, _F_INW:_F_INW + 2 * DI] = raw["in_w"][l].T
    for k in range(K):
        d0 = _F_DIAG + k * DI
        fh[:, d0:d0 + DI][np.arange(DI), np.arange(DI)] = raw["conv_w"][l][:, k]
    fh[:, _F_XPW:_F_XPW + XR] = raw["xp_w"][l].T
    fh[:DR, _F_DTW:_F_DTW + DI] = raw["dt_w"][l].T
    fh[:, _F_OUTW:_F_OUTW + DM] = raw["out_w"][l].T
    for p in range(_N_MM_PAIRS):
        for r in range(4):  # B(s0), B(s1), C(s0), C(s1)
            row = DR + 2 * p + (r & 1) + (DS if r >= 2 else 0)
            c0 = _F_OH + (4 * p + r) * DI
            fh[row, c0:c0 + DI] = 1.0
    fh[:, _F_ID:_F_ID + DI][np.arange(DI), np.arange(DI)] = 1.0
    return fh


def _pack_fv(raw, l):
    fv = np.zeros((DI, _VW), np.float32)
    fv[:, _V_CONVB] = raw["conv_b"][l]
    fv[:, _V_DTB] = raw["dt_b"][l]
    fv[:, _V_D] = raw["D"][l]
    fv[:, _V_ANEG:_V_ANEG + DS] = -np.exp(raw["A_log"][l])
    return fv


def _build_kernel(ctx, tc, u0T, fhs, fvs, xdbl_ds, outs):
    nc = tc.nc
    NCH = 4
    CF = T // NCH  # 512

    const = ctx.enter_context(tc.tile_pool(name="const", bufs=1))
    big = ctx.enter_context(tc.tile_pool(name="big", bufs=1))
    ub = ctx.enter_context(tc.tile_pool(name="ub", bufs=2))
    sl = ctx.enter_context(tc.tile_pool(name="sl", bufs=2))
    bc = ctx.enter_context(tc.tile_pool(name="bc", bufs=4))
    ps1 = ctx.enter_context(tc.tile_pool(name="ps1", bufs=4, space="PSUM"))
    psy = ctx.enter_context(tc.tile_pool(name="psy", bufs=1, space="PSUM"))

    fh = []
    fv = []
    for l in range(NL):
        t = const.tile([DI, _FW], FH, tag=f"fh{l}", name=f"fh{l}")
        nc.sync.dma_start(t[:], fhs[l][:])
        fh.append(t)
        v = const.tile([DI, _VW], FP, tag=f"fv{l}", name=f"fv{l}")
        nc.sync.dma_start(v[:], fvs[l][:])
        fv.append(v)

    u_t = ub.tile([DM, T], FH, tag="u", name="u_in")
    nc.sync.dma_start(u_t[:], u0T[:])

    for l in range(NL):
        h = fh[l]
        v = fv[l]
        in_wT = h[0:DM, _F_INW:_F_INW + 2 * DI]
        xp_wT = h[0:DI, _F_XPW:_F_XPW + XR]
        dt_wT = h[0:DR, _F_DTW:_F_DTW + DI]
        out_wT = h[:, _F_OUTW:_F_OUTW + DM]
        ident = h[:, _F_ID:_F_ID + DI]
        convb = v[:, _V_CONVB:_V_CONVB + 1]
        dt_b = v[:, _V_DTB:_V_DTB + 1]
        Dvec = v[:, _V_D:_V_D + 1]

        # ---- stage 1: projections + conv ----
        xpre = big.tile([DI, T + K - 1], FH, tag="xpre", name=f"xpre{l}")
        zs = big.tile([DI, T], FH, tag="zs", name=f"zs{l}")
        xact = big.tile([DI, T], FH, tag="xact", name=f"xact{l}")
        xdbl = big.tile([XR, T], FH, tag="xdbl", name=f"xdbl{l}")
        ev = big.tile([DI, T], FP, tag="ev", name=f"ev{l}")
        delta16 = big.tile([DI, T], FH, tag="delta", name=f"delta{l}")
        dx16 = big.tile([DI, T], FH, tag="dx", name=f"dx{l}")
        nc.gpsimd.memset(xpre[:, 0:K - 1], 0.0)
        for c in range(NCH):
            cs = slice(c * CF, (c + 1) * CF)
            mmx = ps1.tile([DI, CF], FP, tag="mm", name=f"mmx{l}_{c}")
            nc.tensor.matmul(mmx[:], in_wT[:, 0:DI], u_t[:, cs],
                             start=True, stop=True)
            nc.scalar.activation(xpre[:, K - 1 + c * CF:K - 1 + (c + 1) * CF],
                                 mmx[:], AF.Copy)
            mmz = ps1.tile([DI, CF], FP, tag="mm", name=f"mmz{l}_{c}")
            nc.tensor.matmul(mmz[:], in_wT[:, DI:2 * DI], u_t[:, cs],
                             start=True, stop=True)
            nc.scalar.activation(zs[:, cs], mmz[:], AF.Silu)
        for c in range(NCH):
            cs = slice(c * CF, (c + 1) * CF)
            cps = ps1.tile([DI, CF], FP, tag="mm", name=f"cps{l}_{c}")
            for k in range(K):
                nc.tensor.matmul(cps[:], h[:, _F_DIAG + k * DI:_F_DIAG + (k + 1) * DI],
                                 xpre[:, k + c * CF:k + c * CF + CF],
                                 start=(k == 0), stop=(k == K - 1))
            nc.scalar.activation(xact[:, cs], cps[:], AF.Silu, bias=convb)
        for c in range(NCH):
            cs = slice(c * CF, (c + 1) * CF)
            mmp = ps1.tile([DI, CF], FP, tag="mm", name=f"mmp{l}_{c}")
            nc.tensor.matmul(mmp[0:XR, :], xp_wT, xact[:, cs],
                             start=True, stop=True)
            nc.scalar.activation(xdbl[:, cs], mmp[0:XR, :], AF.Copy)
        # stage the B/C rows in DRAM, pair-interleaved [B2p B2p+1 C2p C2p+1]
        nc.sync.dma_start(xdbl_ds[l][:, 0:2 * T], xdbl[DR:DR + DS, :])
        nc.sync.dma_start(xdbl_ds[l][:, 2 * T:4 * T], xdbl[DR + DS:XR, :])
        for c in range(NCH):
            cs = slice(c * CF, (c + 1) * CF)
            mmd = ps1.tile([DI, CF], FP, tag="mm", name=f"mmd{l}_{c}")
            nc.tensor.matmul(mmd[:], dt_wT, xdbl[0:DR, cs],
                             start=True, stop=True)
            nc.scalar.activation(ev[:, cs], mmd[:], AF.Exp, bias=dt_b)
        nc.scalar.activation(delta16[:], ev[:], AF.Ln, bias=1.0)
        nc.vector.tensor_mul(dx16[:], delta16[:], xact[:])

        # ---- SSM: 8 pairs of states ----
        y_ps = psy.tile([DI, T], FP, tag="y", name=f"y{l}")
        for p in range(DS // 2):
            s0 = 2 * p
            # bcrep layout: [B(s0) | B(s0+1) | C(s0) | C(s0+1)], T cols each
            bcrep = bc.tile([DI, 4 * T], FH, tag="bcrep", name=f"bcrep{l}_{p}")
            if p < _N_MM_PAIRS:
                for r in range(4):
                    oh = h[0:XR, _F_OH + (4 * p + r) * DI:
                           _F_OH + (4 * p + r + 1) * DI]
                    for c in range(NCH):
                        bps = ps1.tile([DI, CF], FP, tag="mm",
                                       name=f"bc{l}_{p}_{r}_{c}")
                        nc.tensor.matmul(bps[:], oh,
                                         xdbl[:, c * CF:(c + 1) * CF],
                                         start=True, stop=True)
                        nc.scalar.activation(
                            bcrep[:, r * T + c * CF:r * T + (c + 1) * CF],
                            bps[:], AF.Copy)
            elif p < 2:
                # head pairs: land the B half first so dBu can start sooner
                bsrc = xdbl_ds[l][p:p + 1, 0:2 * T]
                nc.sync.dma_start(bcrep[:, 0:2 * T],
                                  bsrc.broadcast_to((DI, 2 * T)))
                csrc = xdbl_ds[l][p:p + 1, 2 * T:4 * T]
                nc.sync.dma_start(bcrep[:, 2 * T:4 * T],
                                  csrc.broadcast_to((DI, 2 * T)))
            else:
                src = xdbl_ds[l][p:p + 1, :]
                nc.sync.dma_start(bcrep[:], src.broadcast_to((DI, 4 * T)))
            brep = bcrep[:, 0:2 * T]
            crep = bcrep[:, 2 * T:4 * T]
            dA2 = sl.tile([DI, 2 * T], FH, tag="dA2", name=f"dA2{l}_{p}")
            for q in range(2):
                nc.scalar.activation(dA2[:, q * T:(q + 1) * T], delta16[:],
                                     AF.Exp,
                                     scale=v[:, _V_ANEG + s0 + q:_V_ANEG + s0 + q + 1])
            nc.gpsimd.memset(dA2[:, T:T + 1], 0.0)
            dBu2 = sl.tile([DI, 2 * T], FH, tag="dBu2", name=f"dBu2{l}_{p}")
            nc.vector.tensor_mul(
                dBu2[:].rearrange("p (s t) -> p s t", s=2),
                dx16[:].unsqueeze(1).broadcast_to((DI, 2, T)),
                brep.rearrange("p (s t) -> p s t", s=2))
            hs2 = sl.tile([DI, 2 * T], FH, tag="hs2", name=f"hs2{l}_{p}")
            nc.vector.tensor_tensor_scan(hs2[:], dA2[:], dBu2[:], 0.0,
                                         AX.mult, AX.add)
            hsC2 = sl.tile([DI, 2 * T], FH, tag="hsC2", name=f"hsC2{l}_{p}")
            nc.vector.tensor_mul(hsC2[:], hs2[:], crep[:])
            for q in range(2):
                for c in range(NCH):
                    nc.tensor.matmul(y_ps[:, c * CF:(c + 1) * CF], ident,
                                     hsC2[:, q * T + c * CF:q * T + (c + 1) * CF],
                                     start=(p == 0 and q == 0),
                                     stop=(p == DS // 2 - 1 and q == 1),
                                     skip_group_check=True)

        # ---- tail: y = (yacc + D*x) * silu(z); out projection ----
        y16 = big.tile([DI, T], FH, tag="y16", name=f"y16{l}")
        ydx = big.tile([DI, T], FH, tag="ydx", name=f"ydx{l}")
        yf = big.tile([DI, T], FH, tag="yf", name=f"yf{l}")
        for hf in range(2):
            hs_ = slice(hf * 1024, (hf + 1) * 1024)
            nc.scalar.activation(y16[:, hs_], y_ps[:, hs_], AF.Copy)
            nc.vector.tensor_scalar_mul(ydx[:, hs_], xact[:, hs_], Dvec)
            nc.vector.tensor_add(ydx[:, hs_], ydx[:, hs_], y16[:, hs_])
            nc.vector.tensor_mul(yf[:, hs_], ydx[:, hs_], zs[:, hs_])

        o_t = ub.tile([DM, T], FH, tag="u", name=f"o{l}")
        for c in range(NCH):
            cs = slice(c * CF, (c + 1) * CF)
            omm = ps1.tile([DM, CF], FP, tag="mm", name=f"omm{l}_{c}")
            nc.tensor.matmul(omm[:], out_wT, yf[:, cs], start=True, stop=True)
            nc.scalar.activation(o_t[:, cs], omm[:], AF.Copy)
        nc.sync.dma_start(outs[l][:], o_t[:])
        u_t = o_t


def build_program():
    nc = bacc.Bacc("TRN2", target_bir_lowering=False, debug=False)
    u0T = nc.dram_tensor("u0T", [DM, T], FH, kind="ExternalInput").ap()
    fhs = [nc.dram_tensor(f"fh_l{l}", [DI, _FW], FH,
                          kind="ExternalInput").ap() for l in range(NL)]
    fvs = [nc.dram_tensor(f"fv_l{l}", [DI, _VW], FP,
                          kind="ExternalInput").ap() for l in range(NL)]
    xdbl_ds = [nc.dram_tensor(f"xdbl_d{l}", [DS // 2, 4 * T], FH,
                              kind="Internal").ap() for l in range(NL)]
    outs = [nc.dram_tensor(f"o{l + 1}T", [DM, T], FH,
                           kind="ExternalOutput").ap() for l in range(NL)]
    with tile.TileContext(nc) as tc:
        with ExitStack() as ctx:
            _build_kernel(ctx, tc, u0T, fhs, fvs, xdbl_ds, outs)
    nc.compile()
    return nc


_PROG = None


def _get_prog():
    global _PROG
    if _PROG is None:
        _PROG = build_program()
    return _PROG


def _run_launch(u_list_T, raw, trace=False, trace_kwargs=None):
    """u_list_T: list of 8 arrays (64, 2048) fp16. raw: param dict (np).
    Returns (o1_list, o2_list) fp16 arrays and the raw result."""
    nc = _get_prog()
    fhs = [_pack_fh(raw, l) for l in range(NL)]
    fvs = [_pack_fv(raw, l) for l in range(NL)]
    in_maps = []
    for b in range(8):
        in_maps.append({
            "u0T": np.ascontiguousarray(u_list_T[b], np.float16),
            "fh_l0": fhs[0], "fh_l1": fhs[1],
            "fv_l0": fvs[0], "fv_l1": fvs[1],
        })
    res = bass_utils.run_bass_kernel_spmd(
        nc, in_maps, core_ids=list(range(8)), trace=trace,
        **(trace_kwargs or {}))
    o1 = [res.results[b]["o1T"] for b in range(8)]
    o2 = [res.results[b]["o2T"] for b in range(8)]
    return o1, o2, res


def kernel(**inputs):
    inp = {k: np.asarray(v, np.float32) for k, v in inputs.items()}
    Ms = inp["Ms_feature"]
    Pan = inp["Pan_feature"]
    h = C // 2
    names = ("in_w", "conv_w", "conv_b", "xp_w", "dt_w", "dt_b",
             "A_log", "D", "out_w")
    rawa = {n: inp["a_" + n] for n in names}
    rawb = {n: inp["b_" + n] for n in names}

    cf1 = np.concatenate([Ms[:, :h], Pan[:, h:]], axis=1)
    cf2 = np.concatenate([Pan[:, :h], Ms[:, h:]], axis=1)
    u_list = [cf1[b].T for b in range(B)] + [cf2[b].T for b in range(B)]
    o1, o2, _ = _run_launch(u_list, rawa)
    cf1_1 = np.stack([o1[b].T.astype(np.float32) for b in range(B)])
    cf2_1 = np.stack([o1[B + b].T.astype(np.float32) for b in range(B)])
    cf1_2 = np.stack([o2[b].T.astype(np.float32) for b in range(B)])
    cf2_2 = np.stack([o2[B + b].T.astype(np.float32) for b in range(B)])
    Ms1 = np.maximum((cf1_1 + cf2_1) * 0.5 + Ms, 0.0)
    Ms2 = np.maximum((cf1_2 + cf2_2) * 0.5 + Ms1, 0.0)

    cf3 = np.stack([Pan[:, ::2], Ms2[:, 1::2]], axis=2).reshape(B, C, DM)
    cf4 = np.stack([Ms2[:, ::2], Pan[:, 1::2]], axis=2).reshape(B, C, DM)
    u_list = [cf3[b].T for b in range(B)] + [cf4[b].T for b in range(B)]
    o1, o2, _ = _run_launch(u_list, rawb)
    cf3_1 = np.stack([o1[b].T.astype(np.float32) for b in range(B)])
    cf4_1 = np.stack([o1[B + b].T.astype(np.float32) for b in range(B)])
    cf3_2 = np.stack([o2[b].T.astype(np.float32) for b in range(B)])
    cf4_2 = np.stack([o2[B + b].T.astype(np.float32) for b in range(B)])
    Pan1 = np.maximum((cf3_1 + cf4_1) * 0.5 + Pan, 0.0)
    Pan2 = np.maximum((cf3_2 + cf4_2) * 0.5 + Pan1, 0.0)
    return Ms2, Pan2


# revision 4
# speedup vs baseline: 1.0284x; 1.0073x over previous
"""Trainium2 Bass kernel for nn_CMCI_Mamba — v2 (engine-rebalanced).

Data-parallel over the 2B=8 mamba streams (1 per core), 2 chained layers
per launch, d-major layout (d_inner on partitions, time on free axis).

v2 vs baseline:
- fp16 tiles everywhere (host pre-casts inputs / params, upcasts outputs)
- silu fused into ACT Swish evictions (no sigmoid+mul pairs)
- depthwise conv on PE via diagonal-lhsT accumulated matmuls
- x-projection computed once (all 36 rows); per-state B/C row broadcasts
  are one-hot selection matmuls reading the 36-row tile
- per-state products on DVE; the sum over states runs on PE as
  identity-matmul PSUM accumulation
- scans paired: one tensor_tensor_scan of FD=4096 covers 2 states
  (decay column at the segment boundary zeroed to reset the recurrence)
"""
import sys
import numpy as np
from contextlib import ExitStack

for _p in ("/opt/trn_rl_repo",):
    if _p not in sys.path:
        sys.path.insert(0, _p)

import concourse.bass as bass
import concourse.bacc as bacc
import concourse.tile as tile
from concourse import mybir
from concourse import bass_utils

T, DM, DI, DS, DR, K, NL = 2048, 64, 128, 16, 4, 4, 2
B, C = 4, 2048
XR = DR + 2 * DS  # 36 rows of the x-projection
FP = mybir.dt.float32
FH = mybir.dt.float16
AX = mybir.AluOpType
AF = mybir.ActivationFunctionType

# fp16 param blob column layout, (128, _FW) per layer
_F_INW = 0                    # [0:64, 0:256]   in_wT (x cols 0:128, z cols 128:256)
_F_DIAG = 256                 # [:, 256:768]    conv diag_k (4 x 128)
_F_XPW = 768                  # [0:128, 768:804]  xp_wT (128, 36)
_F_DTW = 804                  # [0:4, 804:932]  dt_wT (4, 128)
_F_OUTW = 932                 # [:, 932:996]    out_wT (128, 64)
_F_OH = 996                   # one-hot B/C selectors for the MM-path pairs
_N_MM_PAIRS = 0               # pairs < this broadcast via PE+ACT, rest via DMA
_F_ID = _F_OH + 4 * _N_MM_PAIRS * 128
_FW = _F_ID + 128

# fp32 vector blob (128, 19): conv_b, dt_b, D, Aneg[16]
_V_CONVB, _V_DTB, _V_D, _V_ANEG = 0, 1, 2, 3
_VW = 19


def _pack_fh(raw, l):
    fh = np.zeros((DI, _FW), np.float16)
    fh[:DM, _F_INW:_F_INW + 2 * DI] = raw["in_w"][l].T
    for k in range(K):
        d0 = _F_DIAG + k * DI
        fh[:, d0:d0 + DI][np.arange(DI), np.arange(DI)] = raw["conv_w"][l][:, k]
    fh[:, _F_XPW:_F_XPW + XR] = raw["xp_w"][l].T
    fh[:DR, _F_DTW:_F_DTW + DI] = raw["dt_w"][l].T
    fh[:, _F_OUTW:_F_OUTW + DM] = raw["out_w"][l].T
    for p in range(_N_MM_PAIRS):
        for r in range(4):  # B(s0), B(s1), C(s0), C(s1)
            row = DR + 2 * p + (r & 1) + (DS if r >= 2 else 0)
            c0 = _F_OH + (4 * p + r) * DI
            fh[row, c0:c0 + DI] = 1.0
    fh[:, _F_ID:_F_ID + DI][np.arange(DI), np.arange(DI)] = 1.0
    return fh


def _pack_fv(raw, l):
    fv = np.zeros((DI, _VW), np.float32)
    fv[:, _V_CONVB] = raw["conv_b"][l]
    fv[:, _V_DTB] = raw["dt_b"][l]
    fv[:, _V_D] = raw["D"][l]
    fv[:, _V_ANEG:_V_ANEG + DS] = -np.exp(raw["A_log"][l])
    return fv


def _build_kernel(ctx, tc, u0T, fhs, fvs, xdbl_ds, outs):
    nc = tc.nc
    NCH = 4
    CF = T // NCH  # 512

    const = ctx.enter_context(tc.tile_pool(name="const", bufs=1))
    big = ctx.enter_context(tc.tile_pool(name="big", bufs=1))
    ub = ctx.enter_context(tc.tile_pool(name="ub", bufs=2))
    sl = ctx.enter_context(tc.tile_pool(name="sl", bufs=2))
    bc = ctx.enter_context(tc.tile_pool(name="bc", bufs=5))
    ps1 = ctx.enter_context(tc.tile_pool(name="ps1", bufs=4, space="PSUM"))
    psy = ctx.enter_context(tc.tile_pool(name="psy", bufs=1, space="PSUM"))

    fh = []
    fv = []
    for l in range(NL):
        t = const.tile([DI, _FW], FH, tag=f"fh{l}", name=f"fh{l}")
        nc.sync.dma_start(t[:], fhs[l][:])
        fh.append(t)
        v = const.tile([DI, _VW], FP, tag=f"fv{l}", name=f"fv{l}")
        nc.sync.dma_start(v[:], fvs[l][:])
        fv.append(v)

    u_t = ub.tile([DM, T], FH, tag="u", name="u_in")
    nc.sync.dma_start(u_t[:], u0T[:])

    for l in range(NL):
        h = fh[l]
        v = fv[l]
        in_wT = h[0:DM, _F_INW:_F_INW + 2 * DI]
        xp_wT = h[0:DI, _F_XPW:_F_XPW + XR]
        dt_wT = h[0:DR, _F_DTW:_F_DTW + DI]
        out_wT = h[:, _F_OUTW:_F_OUTW + DM]
        ident = h[:, _F_ID:_F_ID + DI]
        convb = v[:, _V_CONVB:_V_CONVB + 1]
        dt_b = v[:, _V_DTB:_V_DTB + 1]
        Dvec = v[:, _V_D:_V_D + 1]

        # ---- stage 1: projections + conv ----
        xpre = big.tile([DI, T + K - 1], FH, tag="xpre", name=f"xpre{l}")
        zs = big.tile([DI, T], FH, tag="zs", name=f"zs{l}")
        xact = big.tile([DI, T], FH, tag="xact", name=f"xact{l}")
        xdbl = big.tile([XR, T], FH, tag="xdbl", name=f"xdbl{l}")
        ev = big.tile([DI, T], FP, tag="ev", name=f"ev{l}")
        delta16 = big.tile([DI, T], FH, tag="delta", name=f"delta{l}")
        dx16 = big.tile([DI, T], FH, tag="dx", name=f"dx{l}")
        nc.gpsimd.memset(xpre[:, 0:K - 1], 0.0)
        for c in range(NCH):
            cs = slice(c * CF, (c + 1) * CF)
            mmx = ps1.tile([DI, CF], FP, tag="mm", name=f"mmx{l}_{c}")
            nc.tensor.matmul(mmx[:], in_wT[:, 0:DI], u_t[:, cs],
                             start=True, stop=True)
            nc.scalar.activation(xpre[:, K - 1 + c * CF:K - 1 + (c + 1) * CF],
                                 mmx[:], AF.Copy)
            mmz = ps1.tile([DI, CF], FP, tag="mm", name=f"mmz{l}_{c}")
            nc.tensor.matmul(mmz[:], in_wT[:, DI:2 * DI], u_t[:, cs],
                             start=True, stop=True)
            nc.scalar.activation(zs[:, cs], mmz[:], AF.Silu)
        for c in range(NCH):
            cs = slice(c * CF, (c + 1) * CF)
            cps = ps1.tile([DI, CF], FP, tag="mm", name=f"cps{l}_{c}")
            for k in range(K):
                nc.tensor.matmul(cps[:], h[:, _F_DIAG + k * DI:_F_DIAG + (k + 1) * DI],
                                 xpre[:, k + c * CF:k + c * CF + CF],
                                 start=(k == 0), stop=(k == K - 1))
            nc.scalar.activation(xact[:, cs], cps[:], AF.Silu, bias=convb)
        for c in range(NCH):
            cs = slice(c * CF, (c + 1) * CF)
            mmp = ps1.tile([DI, CF], FP, tag="mm", name=f"mmp{l}_{c}")
            nc.tensor.matmul(mmp[0:XR, :], xp_wT, xact[:, cs],
                             start=True, stop=True)
            nc.scalar.activation(xdbl[:, cs], mmp[0:XR, :], AF.Copy)
        # stage the B rows in DRAM, pair-interleaved [B2p B2p+1 C2p C2p+1]
        # (C rows staged inside the pair loop, after pair 0's B broadcast,
        # so the head B broadcast isn't queued behind them on the rings)
        nc.sync.dma_start(xdbl_ds[l][:, 0:2 * T], xdbl[DR:DR + DS, :])
        for c in range(NCH):
            cs = slice(c * CF, (c + 1) * CF)
            mmd = ps1.tile([DI, CF], FP, tag="mm", name=f"mmd{l}_{c}")
            nc.tensor.matmul(mmd[:], dt_wT, xdbl[0:DR, cs],
                             start=True, stop=True)
            nc.scalar.activation(ev[:, cs], mmd[:], AF.Exp, bias=dt_b)
        nc.scalar.activation(delta16[:], ev[:], AF.Ln, bias=1.0)
        nc.vector.tensor_mul(dx16[:], delta16[:], xact[:])

        # ---- SSM: 8 pairs of states ----
        y_ps = psy.tile([DI, T], FP, tag="y", name=f"y{l}")
        for p in range(DS // 2):
            s0 = 2 * p
            # bcrep layout: [B(s0) | B(s0+1) | C(s0) | C(s0+1)], T cols each
            bcrep = bc.tile([DI, 4 * T], FH, tag="bcrep", name=f"bcrep{l}_{p}")
            if p < _N_MM_PAIRS:
                for r in range(4):
                    oh = h[0:XR, _F_OH + (4 * p + r) * DI:
                           _F_OH + (4 * p + r + 1) * DI]
                    for c in range(NCH):
                        bps = ps1.tile([DI, CF], FP, tag="mm",
                                       name=f"bc{l}_{p}_{r}_{c}")
                        nc.tensor.matmul(bps[:], oh,
                                         xdbl[:, c * CF:(c + 1) * CF],
                                         start=True, stop=True)
                        nc.scalar.activation(
                            bcrep[:, r * T + c * CF:r * T + (c + 1) * CF],
                            bps[:], AF.Copy)
            elif p < 2:
                # head pairs: land the B half first so dBu can start sooner
                bsrc = xdbl_ds[l][p:p + 1, 0:2 * T]
                nc.sync.dma_start(bcrep[:, 0:2 * T],
                                  bsrc.broadcast_to((DI, 2 * T)))
                if p == 0:
                    nc.sync.dma_start(xdbl_ds[l][:, 2 * T:4 * T],
                                      xdbl[DR + DS:XR, :])
                csrc = xdbl_ds[l][p:p + 1, 2 * T:4 * T]
                nc.sync.dma_start(bcrep[:, 2 * T:4 * T],
                                  csrc.broadcast_to((DI, 2 * T)))
            else:
                src = xdbl_ds[l][p:p + 1, :]
                nc.sync.dma_start(bcrep[:], src.broadcast_to((DI, 4 * T)))
            brep = bcrep[:, 0:2 * T]
            crep = bcrep[:, 2 * T:4 * T]
            dA2 = sl.tile([DI, 2 * T], FH, tag="dA2", name=f"dA2{l}_{p}")
            for q in range(2):
                nc.scalar.activation(dA2[:, q * T:(q + 1) * T], delta16[:],
                                     AF.Exp,
                                     scale=v[:, _V_ANEG + s0 + q:_V_ANEG + s0 + q + 1])
            nc.gpsimd.memset(dA2[:, T:T + 1], 0.0)
            dBu2 = sl.tile([DI, 2 * T], FH, tag="dBu2", name=f"dBu2{l}_{p}")
            nc.vector.tensor_mul(
                dBu2[:].rearrange("p (s t) -> p s t", s=2),
                dx16[:].unsqueeze(1).broadcast_to((DI, 2, T)),
                brep.rearrange("p (s t) -> p s t", s=2))
            hs2 = sl.tile([DI, 2 * T], FH, tag="hs2", name=f"hs2{l}_{p}")
            nc.vector.tensor_tensor_scan(hs2[:], dA2[:], dBu2[:], 0.0,
                                         AX.mult, AX.add)
            hsC2 = sl.tile([DI, 2 * T], FH, tag="hsC2", name=f"hsC2{l}_{p}")
            nc.vector.tensor_mul(hsC2[:], hs2[:], crep[:])
            for q in range(2):
                for c in range(NCH):
                    nc.tensor.matmul(y_ps[:, c * CF:(c + 1) * CF], ident,
                                     hsC2[:, q * T + c * CF:q * T + (c + 1) * CF],
                                     start=(p == 0 and q == 0),
                                     stop=(p == DS // 2 - 1 and q == 1),
                                     skip_group_check=True)

        # ---- tail: y = (yacc + D*x) * silu(z); out projection ----
        y16 = big.tile([DI, T], FH, tag="y16", name=f"y16{l}")
        ydx = big.tile([DI, T], FH, tag="ydx", name=f"ydx{l}")
        yf = big.tile([DI, T], FH, tag="yf", name=f"yf{l}")
        for hf in range(2):
            hs_ = slice(hf * 1024, (hf + 1) * 1024)
            nc.scalar.activation(y16[:, hs_], y_ps[:, hs_], AF.Copy)
            nc.vector.tensor_scalar_mul(ydx[:, hs_], xact[:, hs_], Dvec)
            nc.vector.tensor_add(ydx[:, hs_], ydx[:, hs_], y16[:, hs_])
            nc.vector.tensor_mul(yf[:, hs_], ydx[:, hs_], zs[:, hs_])

        o_t = ub.tile([DM, T], FH, tag="u", name=f"o{l}")
        for c in range(NCH):
            cs = slice(c * CF, (c + 1) * CF)
            omm = ps1.tile([DM, CF], FP, tag="mm", name=f"omm{l}_{c}")
            nc.tensor.matmul(omm[:], out_wT, yf[:, cs], start=True, stop=True)
            nc.scalar.activation(o_t[:, cs], omm[:], AF.Copy)
        nc.sync.dma_start(outs[l][:], o_t[:])
        u_t = o_t


def build_program():
    nc = bacc.Bacc("TRN2", target_bir_lowering=False, debug=False)
    u0T = nc.dram_tensor("u0T", [DM, T], FH, kind="ExternalInput").ap()
    fhs = [nc.dram_tensor(f"fh_l{l}", [DI, _FW], FH,
                          kind="ExternalInput").ap() for l in range(NL)]
    fvs = [nc.dram_tensor(f"fv_l{l}", [DI, _VW], FP,
                          kind="ExternalInput").ap() for l in range(NL)]
    xdbl_ds = [nc.dram_tensor(f"xdbl_d{l}", [DS // 2, 4 * T], FH,
                              kind="Internal").ap() for l in range(NL)]
    outs = [nc.dram_tensor(f"o{l + 1}T", [DM, T], FH,
                           kind="ExternalOutput").ap() for l in range(NL)]
    with tile.TileContext(nc) as tc:
        with ExitStack() as ctx:
            _build_kernel(ctx, tc, u0T, fhs, fvs, xdbl_ds, outs)
    nc.compile()
    return nc


_PROG = None


def _get_prog():
    global _PROG
    if _PROG is None:
        _PROG = build_program()
    return _PROG


def _run_launch(u_list_T, raw, trace=False, trace_kwargs=None):
    """u_list_T: list of 8 arrays (64, 2048) fp16. raw: param dict (np).
    Returns (o1_list, o2_list) fp16 arrays and the raw result."""
    nc = _get_prog()
    fhs = [_pack_fh(raw, l) for l in range(NL)]
    fvs = [_pack_fv(raw, l) for l in range(NL)]
    in_maps = []
    for b in range(8):
        in_maps.append({
            "u0T": np.ascontiguousarray(u_list_T[b], np.float16),
            "fh_l0": fhs[0], "fh_l1": fhs[1],
            "fv_l0": fvs[0], "fv_l1": fvs[1],
        })
    res = bass_utils.run_bass_kernel_spmd(
        nc, in_maps, core_ids=list(range(8)), trace=trace,
        **(trace_kwargs or {}))
    o1 = [res.results[b]["o1T"] for b in range(8)]
    o2 = [res.results[b]["o2T"] for b in range(8)]
    return o1, o2, res


def kernel(**inputs):
    inp = {k: np.asarray(v, np.float32) for k, v in inputs.items()}
    Ms = inp["Ms_feature"]
    Pan = inp["Pan_feature"]
    h = C // 2
    names = ("in_w", "conv_w", "conv_b", "xp_w", "dt_w", "dt_b",
             "A_log", "D", "out_w")
    rawa = {n: inp["a_" + n] for n in names}
    rawb = {n: inp["b_" + n] for n in names}

    cf1 = np.concatenate([Ms[:, :h], Pan[:, h:]], axis=1)
    cf2 = np.concatenate([Pan[:, :h], Ms[:, h:]], axis=1)
    u_list = [cf1[b].T for b in range(B)] + [cf2[b].T for b in range(B)]
    o1, o2, _ = _run_launch(u_list, rawa)
    cf1_1 = np.stack([o1[b].T.astype(np.float32) for b in range(B)])
    cf2_1 = np.stack([o1[B + b].T.astype(np.float32) for b in range(B)])
    cf1_2 = np.stack([o2[b].T.astype(np.float32) for b in range(B)])
    cf2_2 = np.stack([o2[B + b].T.astype(np.float32) for b in range(B)])
    Ms1 = np.maximum((cf1_1 + cf2_1) * 0.5 + Ms, 0.0)
    Ms2 = np.maximum((cf1_2 + cf2_2) * 0.5 + Ms1, 0.0)

    cf3 = np.stack([Pan[:, ::2], Ms2[:, 1::2]], axis=2).reshape(B, C, DM)
    cf4 = np.stack([Ms2[:, ::2], Pan[:, 1::2]], axis=2).reshape(B, C, DM)
    u_list = [cf3[b].T for b in range(B)] + [cf4[b].T for b in range(B)]
    o1, o2, _ = _run_launch(u_list, rawb)
    cf3_1 = np.stack([o1[b].T.astype(np.float32) for b in range(B)])
    cf4_1 = np.stack([o1[B + b].T.astype(np.float32) for b in range(B)])
    cf3_2 = np.stack([o2[b].T.astype(np.float32) for b in range(B)])
    cf4_2 = np.stack([o2[B + b].T.astype(np.float32) for b in range(B)])
    Pan1 = np.maximum((cf3_1 + cf4_1) * 0.5 + Pan, 0.0)
    Pan2 = np.maximum((cf3_2 + cf4_2) * 0.5 + Pan1, 0.0)
    return Ms2, Pan2
